# revision 7
# baseline (speedup 1.0000x reference)
"""GCN encoder (3x GCNConv, cached symmetric norm) on 8 Trainium2 NeuronCores.

Formulation: with dinv = deg^-1/2 (deg counts in-edges + self-loop), each
propagation is  y = Dinv * S(Dinv * z) + b  where S is a plain gather +
scatter-add over edges (incl. self-loops).  Layers 2 and 3 share input h and
the adjacency, so W2|W3 are concatenated -> only 2 propagations of 64-wide
features.

Sharding: nodes are contiguously sharded 12500/core (padded to 12544).  Each
core computes u = dinv*(x@W1) for its shard, an AllGather builds the full
node-feature table (100352 rows, 44 zero rows per shard used as gather
padding targets), and each core then reduces messages for its own 12544
destination slots.

The per-edge gather uses dma_gather (int16 indices, max 32767), so the table
is processed in 4 quarters of 25088 rows.  Per quarter, destinations are
sorted by per-quarter in-degree and laid out in ELL windows of 128; one
dma_gather fetches [128, R, 64] slabs (row i -> partition i%128), a DVE
strided reduce sums the R slots, giving a per-quarter partial table Y_q.  A
second 4-row gather+reduce combines the quarters in the total-degree order.
"""

import time
import numpy as np

import concourse.bass as bass
import concourse.tile as tile
from concourse import bacc, mybir
from concourse import bass_utils
from concourse.masks import make_identity

N = 100000
IN_CH, HID, EMB = 128, 64, 32
NC = 8
SH = 12500            # real nodes per core
SHP = 12544           # padded shard rows (98 * 128)
NW = 98               # windows per core
QROWS = 2 * SHP       # rows per table quarter (25088 < 32768)
PADIDX = 12500        # a guaranteed-zero row inside each quarter
L2_CHUNK = 14         # windows per level-2 chunk
L1_SLOT_BUDGET = 96   # max gathered slots per level-1 chunk
L1_MAX_WIN = 28       # max windows per level-1 chunk

f32 = mybir.dt.float32
i16 = mybir.dt.int16


# ----------------------------------------------------------------------------
# host-side preparation
# ----------------------------------------------------------------------------

def _prepare(x, edges):
    """Build the common window schedule and per-core input arrays."""
    x = np.asarray(x, dtype=np.float32)
    src = np.asarray(edges[0], dtype=np.int64)
    dst = np.asarray(edges[1], dtype=np.int64)

    allsrc = np.concatenate([src, np.arange(N, dtype=np.int64)])
    alldst = np.concatenate([dst, np.arange(N, dtype=np.int64)])
    deg = np.bincount(alldst, minlength=N)
    dinv = (1.0 / np.sqrt(deg.astype(np.float64))).astype(np.float32)

    equar = allsrc // (2 * SH)            # source quarter of each message
    ecore = alldst // SH                  # destination core of each message

    # per-(dst, quarter) message counts
    kdq = np.bincount(alldst * 4 + equar, minlength=4 * N).reshape(N, 4)

    # total-degree order (D-order) per core
    rank = np.empty(N, np.int64)
    node_at = np.full((NC, SHP), -1, np.int64)
    for c in range(NC):
        nodes = np.arange(c * SH, (c + 1) * SH)
        order = np.argsort(-deg[nodes], kind="stable")
        rank[nodes[order]] = np.arange(SH)
        node_at[c, :SH] = nodes[order]

    # per-quarter orders and the common window schedule
    rank_q = np.empty((4, N), np.int64)
    Rs = np.zeros((4, NW), np.int64)
    for c in range(NC):
        nodes = np.arange(c * SH, (c + 1) * SH)
        for q in range(4):
            order = np.argsort(-kdq[nodes, q], kind="stable")
            rank_q[q, nodes[order]] = np.arange(SH)
            kk = kdq[nodes[order], q]
            Rs[q] = np.maximum(Rs[q], kk[::128][:NW])
    Rs = np.maximum(Rs, 1)
    cumR = np.concatenate([np.zeros((4, 1), np.int64), np.cumsum(Rs, 1)], 1)

    # level-1 ELL index arrays: per (core, quarter): [sum(Rs[q]), 128] int16
    # value = row index within the quarter of the table
    tloc = ((np.arange(N) // SH) % 2) * SHP + rank  # quarter-local table row
    ell = [[np.full((int(cumR[q, -1]), 128), PADIDX, np.int16)
            for q in range(4)] for _ in range(NC)]
    rq_of_msg = rank_q[equar, alldst]     # dst's rank within (its core, src quarter)
    sortkey = (ecore * 4 + equar) * SHP + rq_of_msg
    ordix = np.argsort(sortkey, kind="stable")
    sk = sortkey[ordix]
    starts = np.r_[0, np.flatnonzero(np.diff(sk)) + 1]
    counts = np.diff(np.r_[starts, len(sk)])
    slot = np.arange(len(sk)) - np.repeat(starts, counts)
    ms = allsrc[ordix]
    w_of = rq_of_msg[ordix] // 128
    p_of = rq_of_msg[ordix] % 128
    c_of = ecore[ordix]
    q_of = equar[ordix]
    rowbase = cumR[q_of, w_of] + slot
    vals = tloc[ms].astype(np.int16)
    for c in range(NC):
        mc = c_of == c
        for q in range(4):
            m = mc & (q_of == q)
            ell[c][q][rowbase[m], p_of[m]] = vals[m]

    # level-1 chunking (common across cores): greedy by slot budget
    l1chunks = []  # (q, w0, nw, slot_off, nslots)
    for q in range(4):
        w = 0
        while w < NW:
            w0 = w
            s0 = cumR[q, w0]
            while (w < NW and (cumR[q, w + 1] - s0) <= L1_SLOT_BUDGET
                   and (w - w0) < L1_MAX_WIN):
                w += 1
            if w == w0:
                w += 1  # single window exceeding budget
            l1chunks.append((q, w0, w - w0, int(s0), int(cumR[q, w] - s0)))

    # pack level-1 indices: columns of a [16, GL1] int16 array
    col_off = []
    off = 0
    segs = [[None] * len(l1chunks) for _ in range(NC)]
    for ci, (q, w0, nw, s0, ns) in enumerate(l1chunks):
        n = ns * 128
        col_off.append(off)
        for c in range(NC):
            flat = ell[c][q][s0:s0 + ns, :].reshape(-1)      # i = s*128+p
            segs[c][ci] = flat.reshape(-1, 16).T             # [16, n/16]
        off += n // 16
    GL1 = off
    gell = np.empty((NC, 16, GL1), np.int16)
    for c in range(NC):
        for ci, (q, w0, nw, s0, ns) in enumerate(l1chunks):
            n16 = (ns * 128) // 16
            gell[c][:, col_off[ci]:col_off[ci] + n16] = segs[c][ci]

    # level-2 indices: for D-rank i, the row of Y_q holding its partial sum
    l2 = np.full((NC, 4, SHP), PADIDX, np.int64)
    for c in range(NC):
        real = node_at[c, :SH]
        for q in range(4):
            l2[c, q, :SH] = rank_q[q, real]
    l2chunks = []  # (w0, nw)
    w = 0
    while w < NW:
        nw = min(L2_CHUNK, NW - w)
        l2chunks.append((w, nw))
        w += nw
    col2_off = []
    off = 0
    for (w0, nw) in l2chunks:
        for q in range(4):
            col2_off.append(off)
            off += (nw * 128) // 16
    GL2 = off
    gl2 = np.empty((NC, 16, GL2), np.int16)
    k = 0
    for (w0, nw) in l2chunks:
        for q in range(4):
            o = col2_off[k]
            n16 = (nw * 128) // 16
            for c in range(NC):
                flat = l2[c, q, w0 * 128:(w0 + nw) * 128].astype(np.int16)
                gl2[c][:, o:o + n16] = flat.reshape(-1, 16).T
            k += 1

    # per-core dense inputs
    xT = np.zeros((NC, IN_CH, SHP), np.float32)
    dinvs = np.zeros((NC, 128, NW), np.float32)
    for c in range(NC):
        xT[c, :, :SH] = x[node_at[c, :SH]].T
        dv = np.zeros(SHP, np.float32)
        dv[:SH] = dinv[node_at[c, :SH]]
        dinvs[c] = dv.reshape(NW, 128).T

    sched = dict(
        Rs=Rs, cumR=cumR, l1chunks=l1chunks, col_off=col_off, GL1=GL1,
        l2chunks=l2chunks, col2_off=col2_off, GL2=GL2,
    )
    arrays = dict(xT=xT, dinvs=dinvs, gell=gell, gl2=gl2, node_at=node_at)
    return sched, arrays


def _sched_key(sched):
    import hashlib
    h = hashlib.sha256()
    h.update(sched["Rs"].tobytes())
    h.update(np.asarray(sched["l1chunks"], np.int64).tobytes())
    h.update(np.asarray(sched["l2chunks"], np.int64).tobytes())
    return h.hexdigest()


# ----------------------------------------------------------------------------
# bass module
# ----------------------------------------------------------------------------

def _build(sched, reps=1):
    Relu = mybir.ActivationFunctionType.Relu
    Copy = mybir.ActivationFunctionType.Copy
    X = mybir.AxisListType.X
    ADD = mybir.AluOpType.add
    MULT = mybir.AluOpType.mult

    Rs, cumR = sched["Rs"], sched["cumR"]
    l1chunks, col_off = sched["l1chunks"], sched["col_off"]
    l2chunks, col2_off = sched["l2chunks"], sched["col2_off"]
    GL1, GL2 = sched["GL1"], sched["GL2"]
    max_ns = max(ch[4] for ch in l1chunks)
    max_l1w = max(ch[2] for ch in l1chunks)

    nc = bacc.Bacc("TRN2", target_bir_lowering=False, debug=False,
                   num_devices=NC)

    xT_d = nc.dram_tensor("xT", [IN_CH, SHP], f32, kind="ExternalInput")
    w1_d = nc.dram_tensor("w1", [IN_CH, HID], f32, kind="ExternalInput")
    w23_d = nc.dram_tensor("w23", [HID, 2 * EMB], f32, kind="ExternalInput")
    b1_d = nc.dram_tensor("b1r", [128, HID], f32, kind="ExternalInput")
    b23_d = nc.dram_tensor("b23r", [128, 2 * EMB], f32, kind="ExternalInput")
    dinv_d = nc.dram_tensor("dinvs", [128, NW], f32, kind="ExternalInput")
    gell_d = nc.dram_tensor("gell", [128, GL1], i16, kind="ExternalInput")
    gl2_d = nc.dram_tensor("gl2", [128, GL2], i16, kind="ExternalInput")
    out_d = nc.dram_tensor("out", [SHP, HID], f32, kind="ExternalOutput")

    with tile.TileContext(nc) as tc:
        tc.race_detector_enabled = False
        with (
            tc.tile_pool(name="const", bufs=1) as cpool,
            tc.tile_pool(name="xslab", bufs=2) as xpool,
            tc.tile_pool(name="stage", bufs=2) as spool,
            tc.tile_pool(name="g1", bufs=2) as gpool,
            tc.tile_pool(name="acc", bufs=3) as apool,
            tc.tile_pool(name="g2", bufs=2) as g2pool,
            tc.tile_pool(name="red", bufs=2) as rpool,
            tc.tile_pool(name="hts", bufs=3) as hpool,
            tc.tile_pool(name="psA", bufs=2, space="PSUM") as ppA,
            tc.tile_pool(name="psT", bufs=2, space="PSUM") as ppT,
            tc.tile_pool(name="psZ", bufs=2, space="PSUM") as ppZ,
            tc.tile_pool(name="dram", bufs=1, space="DRAM") as dpool,
        ):
            w1s = cpool.tile([IN_CH, HID], f32)
            nc.sync.dma_start(out=w1s[:, :], in_=w1_d[:, :])
            w23s = cpool.tile([HID, 2 * EMB], f32)
            nc.sync.dma_start(out=w23s[:, :], in_=w23_d[:, :])
            b1s = cpool.tile([128, HID], f32)
            nc.sync.dma_start(out=b1s[:, :], in_=b1_d[:, :])
            b23s = cpool.tile([128, 2 * EMB], f32)
            nc.sync.dma_start(out=b23s[:, :], in_=b23_d[:, :])
            dinvs = cpool.tile([128, NW], f32)
            nc.sync.dma_start(out=dinvs[:, :], in_=dinv_d[:, :])
            gells = cpool.tile([128, GL1], i16)
            nc.sync.dma_start(out=gells[:, :], in_=gell_d[:, :])
            gl2s = cpool.tile([128, GL2], i16)
            nc.sync.dma_start(out=gl2s[:, :], in_=gl2_d[:, :])
            ident = cpool.tile([128, 128], f32)
            make_identity(nc, ident[:, :])

            u1c = dpool.tile([SHP, HID], f32, tag="u1c")
            u2c = dpool.tile([SHP, HID], f32, tag="u2c")
            table1 = dpool.tile([NC * SHP, HID], f32, tag="table1")
            table2 = dpool.tile([NC * SHP, HID], f32, tag="table2")
            Y = [[dpool.tile([SHP, HID], f32, name=f"Y{p}{q}", tag=f"Y{p}{q}")
                  for q in range(4)] for p in range(2)]
            # Pre-assign DRAM addresses: the scheduling-time trace sim
            # otherwise sees every Shared tile at addr 0 and flags the two
            # collectives as a double write to one tensor.
            from concourse.tile_scheduler import allocate_dram_tiles
            allocate_dram_tiles(tc.tiles, nc)

            def propagation(prop, table, bias, relu, dest):
                # level 1: per-quarter ELL gather + window reduce -> Y
                for (q, w0, nwc, s0, ns) in l1chunks:
                    ci = l1chunks.index((q, w0, nwc, s0, ns))
                    n = ns * 128
                    G = gpool.tile([128, max_ns, HID], f32, tag="g1")
                    nc.gpsimd.dma_gather(
                        G[:, :ns, :],
                        table[q * QROWS:(q + 1) * QROWS, :],
                        gells[:, col_off[ci]:col_off[ci] + n // 16],
                        n, n, HID, single_packet=False,
                    )
                    accc = apool.tile([128, max_l1w * HID], f32, tag="acc")
                    for i in range(nwc):
                        off = int(cumR[q, w0 + i] - s0)
                        R = int(Rs[q, w0 + i])
                        nc.vector.tensor_reduce(
                            out=accc[:, i * HID:(i + 1) * HID],
                            in_=G[:, off:off + R, :].transpose([0, 2, 1]),
                            axis=X, op=ADD,
                        )
                    nc.sync.dma_start(
                        out=Y[prop][q][w0 * 128:(w0 + nwc) * 128, :]
                            .rearrange("(a p) c -> p a c", p=128),
                        in_=accc[:, :nwc * HID]
                            .rearrange("p (a c) -> p a c", c=HID),
                    )

                # level 2: combine quarters, epilogue
                k = 0
                for (w0, nw) in l2chunks:
                    G2 = g2pool.tile([128, 4, L2_CHUNK, HID], f32, tag="g2")
                    for q in range(4):
                        n = nw * 128
                        nc.gpsimd.dma_gather(
                            G2[:, q, :nw, :],
                            Y[prop][q][:, :],
                            gl2s[:, col2_off[k]:col2_off[k] + n // 16],
                            n, n, HID, single_packet=False,
                        )
                        k += 1
                    red = rpool.tile([128, L2_CHUNK * HID], f32, tag="red")
                    nc.vector.tensor_reduce(
                        out=red[:, :nw * HID],
                        in_=G2[:, :, :nw, :].transpose([0, 2, 3, 1]),
                        axis=X, op=ADD,
                    )
                    rv = red[:, :nw * HID].rearrange("p (a c) -> p a c", c=HID)
                    nc.vector.tensor_tensor(
                        out=rv, in0=rv,
                        in1=dinvs[:, w0:w0 + nw].unsqueeze(2)
                            .to_broadcast([128, nw, HID]),
                        op=MULT,
                    )
                    nc.vector.tensor_tensor(
                        out=rv, in0=rv,
                        in1=bias[:, :].unsqueeze(1)
                            .to_broadcast([128, nw, HID]),
                        op=ADD,
                    )
                    if relu:
                        h = hpool.tile([128, L2_CHUNK * HID], f32, tag="h")
                        nc.scalar.activation(
                            out=h[:, :nw * HID], in_=red[:, :nw * HID],
                            func=Relu)
                        ust = spool.tile([128, L2_CHUNK, HID], f32, tag="ust")
                        for i in range(nw):
                            hT = ppT.tile([HID, 128], f32, tag="psT")
                            nc.tensor.transpose(
                                out=hT[:, :],
                                in_=h[:, i * HID:(i + 1) * HID],
                                identity=ident[:, :])
                            hTs = hpool.tile([HID, 128], f32, tag="hts")
                            nc.scalar.copy(out=hTs[:, :], in_=hT[:, :])
                            z2 = ppZ.tile([128, 2 * EMB], f32, tag="psZ")
                            nc.tensor.matmul(
                                out=z2[:, :], lhsT=hTs[:, :], rhs=w23s[:, :],
                                start=True, stop=True)
                            nc.scalar.activation(
                                out=ust[:, i, :], in_=z2[:, :], func=Copy,
                                scale=dinvs[:, w0 + i:w0 + i + 1])
                        nc.sync.dma_start(
                            out=dest[w0 * 128:(w0 + nw) * 128, :]
                                .rearrange("(a p) c -> p a c", p=128),
                            in_=ust[:, :nw, :],
                        )
                    else:
                        nc.sync.dma_start(
                            out=dest[w0 * 128:(w0 + nw) * 128, :]
                                .rearrange("(a p) c -> p a c", p=128),
                            in_=rv,
                        )

            for _rep in range(reps):
                # phase A: u1 = dinv * (x @ W1), shard-local
                TPS = 14  # tiles per slab
                for sl in range(NW // TPS):
                    xsl = xpool.tile([IN_CH, TPS * 128], f32, tag="xslab")
                    nc.sync.dma_start(
                        out=xsl[:, :],
                        in_=xT_d[:, sl * TPS * 128:(sl + 1) * TPS * 128])
                    st = spool.tile([128, TPS, HID], f32, tag="stA")
                    for i in range(TPS):
                        t = sl * TPS + i
                        ps = ppA.tile([128, HID], f32, tag="psA")
                        nc.tensor.matmul(
                            out=ps[:, :], lhsT=xsl[:, i * 128:(i + 1) * 128],
                            rhs=w1s[:, :], start=True, stop=True)
                        nc.vector.tensor_scalar_mul(
                            st[:, i, :], ps[:, :], dinvs[:, t:t + 1])
                    nc.sync.dma_start(
                        out=u1c[sl * TPS * 128:(sl + 1) * TPS * 128, :]
                            .rearrange("(a p) c -> p a c", p=128),
                        in_=st[:, :, :],
                    )

                nc.gpsimd.collective_compute(
                    "AllGather", mybir.AluOpType.bypass,
                    ins=[u1c[:, :]], outs=[table1[:, :]],
                    replica_groups=[list(range(NC))],
                )
                propagation(0, table1, b1s, True, u2c)
                nc.gpsimd.collective_compute(
                    "AllGather", mybir.AluOpType.bypass,
                    ins=[u2c[:, :]], outs=[table2[:, :]],
                    replica_groups=[list(range(NC))],
                )
                propagation(1, table2, b23s, False, out_d)

    nc.compile()
    return nc


_CACHE = {}


def _get_module(sched, reps=1):
    key = (_sched_key(sched), reps)
    if key not in _CACHE:
        _CACHE[key] = _build(sched, reps)
    return _CACHE[key]


def _in_maps(arrays, W1, b1, W2, b2, W3, b3):
    W1 = np.asarray(W1, np.float32)
    w23 = np.concatenate(
        [np.asarray(W2, np.float32), np.asarray(W3, np.float32)], axis=1)
    b1r = np.tile(np.asarray(b1, np.float32)[None, :], (128, 1))
    b23r = np.tile(
        np.concatenate([np.asarray(b2, np.float32),
                        np.asarray(b3, np.float32)])[None, :], (128, 1))
    maps = []
    for c in range(NC):
        maps.append({
            "xT": np.ascontiguousarray(arrays["xT"][c]),
            "w1": W1,
            "w23": w23,
            "b1r": b1r,
            "b23r": b23r,
            "dinvs": np.ascontiguousarray(arrays["dinvs"][c]),
            "gell": np.ascontiguousarray(np.tile(arrays["gell"][c], (8, 1))),
            "gl2": np.ascontiguousarray(np.tile(arrays["gl2"][c], (8, 1))),
        })
    return maps


def kernel(x, edges, W1, b1, W2, b2, W3, b3):
    sched, arrays = _prepare(x, edges)
    nc = _get_module(sched, reps=1)
    maps = _in_maps(arrays, W1, b1, W2, b2, W3, b3)
    res = bass_utils.run_bass_kernel_spmd(nc, maps, core_ids=list(range(NC)))
    out2 = np.empty((N, EMB), np.float32)
    out3 = np.empty((N, EMB), np.float32)
    node_at = arrays["node_at"]
    for c in range(NC):
        r = res.results[c]["out"]
        out2[node_at[c, :SH]] = r[:SH, :EMB]
        out3[node_at[c, :SH]] = r[:SH, EMB:]
    return out2, out3


def benchmark(x, edges, W1, b1, W2, b2, W3, b3, reps=4, iters=3):
    """Estimate on-HW exec time via the rep-delta trick: identical NEFF,
    body repeated `reps` times; transfers are identical, so
    (wall(reps) - wall(1)) / (reps - 1) ~= one body execution."""
    sched, arrays = _prepare(x, edges)
    maps = _in_maps(arrays, W1, b1, W2, b2, W3, b3)
    nc1 = _get_module(sched, reps=1)
    ncR = _get_module(sched, reps=reps)
    cores = list(range(NC))

    def timeit(mod):
        best = float("inf")
        for _ in range(iters):
            t0 = time.time()
            bass_utils.run_bass_kernel_spmd(mod, maps, core_ids=cores)
            best = min(best, time.time() - t0)
        return best

    timeit(nc1)  # warm both paths
    t1 = timeit(nc1)
    tR = timeit(ncR)
    return (tR - t1) / (reps - 1) * 1e9


# revision 8
# speedup vs baseline: 1.2392x; 1.2392x over previous
"""GCN encoder (3x GCNConv, cached symmetric norm) on 8 Trainium2 NeuronCores.

Formulation: with dinv = deg^-1/2 (deg counts in-edges + self-loop), each
propagation is  y = Dinv * S(Dinv * z) + b  where S is a plain gather +
scatter-add over edges (incl. self-loops).  Layers 2 and 3 share input h and
the adjacency, so W2|W3 are concatenated -> only 2 propagations of 64-wide
features.

Sharding: nodes are contiguously sharded 12500/core (padded to 12544).  Each
core computes u = dinv*(x@W1) for its shard, an AllGather builds the full
node-feature table (100352 rows, 44 zero rows per shard used as gather
padding targets), and each core then reduces messages for its own 12544
destination slots.

The per-edge gather uses dma_gather (int16 indices, max 32767), so the table
is processed in 4 quarters of 25088 rows.  Per quarter, destinations are
sorted by per-quarter in-degree and laid out in ELL windows of 128; one
dma_gather fetches [128, R, 64] slabs (row i -> partition i%128), a DVE
strided reduce sums the R slots, giving a per-quarter partial table Y_q.  A
second 4-row gather+reduce combines the quarters in the total-degree order.
"""

import time
import numpy as np

import concourse.bass as bass
import concourse.tile as tile
from concourse import bacc, mybir
from concourse import bass_utils
from concourse.masks import make_identity

N = 100000
IN_CH, HID, EMB = 128, 64, 32
NC = 8
SH = 12500            # real nodes per core
SHP = 12544           # padded shard rows (98 * 128)
NW = 98               # windows per core
QROWS = 2 * SHP       # rows per table quarter (25088 < 32768)
PADIDX = 12500        # a guaranteed-zero row inside each quarter
L2_CHUNK = 14         # windows per level-2 chunk
L1_SLOT_BUDGET = 96   # max gathered slots per level-1 chunk
L1_MAX_WIN = 28       # max windows per level-1 chunk

f32 = mybir.dt.float32
i16 = mybir.dt.int16


# ----------------------------------------------------------------------------
# host-side preparation
# ----------------------------------------------------------------------------

def _prepare(x, edges):
    """Build the common window schedule and per-core input arrays."""
    x = np.asarray(x, dtype=np.float32)
    src = np.asarray(edges[0], dtype=np.int64)
    dst = np.asarray(edges[1], dtype=np.int64)

    allsrc = np.concatenate([src, np.arange(N, dtype=np.int64)])
    alldst = np.concatenate([dst, np.arange(N, dtype=np.int64)])
    deg = np.bincount(alldst, minlength=N)
    dinv = (1.0 / np.sqrt(deg.astype(np.float64))).astype(np.float32)

    equar = allsrc // (2 * SH)            # source quarter of each message
    ecore = alldst // SH                  # destination core of each message

    # per-(dst, quarter) message counts
    kdq = np.bincount(alldst * 4 + equar, minlength=4 * N).reshape(N, 4)

    # total-degree order (D-order) per core
    rank = np.empty(N, np.int64)
    node_at = np.full((NC, SHP), -1, np.int64)
    for c in range(NC):
        nodes = np.arange(c * SH, (c + 1) * SH)
        order = np.argsort(-deg[nodes], kind="stable")
        rank[nodes[order]] = np.arange(SH)
        node_at[c, :SH] = nodes[order]

    # per-quarter orders and the common window schedule
    rank_q = np.empty((4, N), np.int64)
    Rs = np.zeros((4, NW), np.int64)
    for c in range(NC):
        nodes = np.arange(c * SH, (c + 1) * SH)
        for q in range(4):
            order = np.argsort(-kdq[nodes, q], kind="stable")
            rank_q[q, nodes[order]] = np.arange(SH)
            kk = kdq[nodes[order], q]
            Rs[q] = np.maximum(Rs[q], kk[::128][:NW])
    Rs = np.maximum(Rs, 1)
    cumR = np.concatenate([np.zeros((4, 1), np.int64), np.cumsum(Rs, 1)], 1)

    # level-1 ELL index arrays: per (core, quarter): [sum(Rs[q]), 128] int16
    # value = row index within the quarter of the table
    tloc = ((np.arange(N) // SH) % 2) * SHP + rank  # quarter-local table row
    ell = [[np.full((int(cumR[q, -1]), 128), PADIDX, np.int16)
            for q in range(4)] for _ in range(NC)]
    rq_of_msg = rank_q[equar, alldst]     # dst's rank within (its core, src quarter)
    sortkey = (ecore * 4 + equar) * SHP + rq_of_msg
    ordix = np.argsort(sortkey, kind="stable")
    sk = sortkey[ordix]
    starts = np.r_[0, np.flatnonzero(np.diff(sk)) + 1]
    counts = np.diff(np.r_[starts, len(sk)])
    slot = np.arange(len(sk)) - np.repeat(starts, counts)
    ms = allsrc[ordix]
    w_of = rq_of_msg[ordix] // 128
    p_of = rq_of_msg[ordix] % 128
    c_of = ecore[ordix]
    q_of = equar[ordix]
    rowbase = cumR[q_of, w_of] + slot
    vals = tloc[ms].astype(np.int16)
    for c in range(NC):
        mc = c_of == c
        for q in range(4):
            m = mc & (q_of == q)
            ell[c][q][rowbase[m], p_of[m]] = vals[m]

    # level-1 chunking (common across cores): greedy by slot budget
    l1chunks = []  # (q, w0, nw, slot_off, nslots)
    for q in range(4):
        w = 0
        while w < NW:
            w0 = w
            s0 = cumR[q, w0]
            while (w < NW and (cumR[q, w + 1] - s0) <= L1_SLOT_BUDGET
                   and (w - w0) < L1_MAX_WIN):
                w += 1
            if w == w0:
                w += 1  # single window exceeding budget
            l1chunks.append((q, w0, w - w0, int(s0), int(cumR[q, w] - s0)))

    # pack level-1 indices: columns of a [16, GL1] int16 array
    col_off = []
    off = 0
    segs = [[None] * len(l1chunks) for _ in range(NC)]
    for ci, (q, w0, nw, s0, ns) in enumerate(l1chunks):
        n = ns * 128
        col_off.append(off)
        for c in range(NC):
            flat = ell[c][q][s0:s0 + ns, :].reshape(-1)      # i = s*128+p
            segs[c][ci] = flat.reshape(-1, 16).T             # [16, n/16]
        off += n // 16
    GL1 = off
    gell = np.empty((NC, 16, GL1), np.int16)
    for c in range(NC):
        for ci, (q, w0, nw, s0, ns) in enumerate(l1chunks):
            n16 = (ns * 128) // 16
            gell[c][:, col_off[ci]:col_off[ci] + n16] = segs[c][ci]

    # level-2 indices: for D-rank i, the row of Y_q holding its partial sum
    l2 = np.full((NC, 4, SHP), PADIDX, np.int64)
    for c in range(NC):
        real = node_at[c, :SH]
        for q in range(4):
            l2[c, q, :SH] = rank_q[q, real]
    l2chunks = []  # (w0, nw)
    w = 0
    while w < NW:
        nw = min(L2_CHUNK, NW - w)
        l2chunks.append((w, nw))
        w += nw
    col2_off = []
    off = 0
    for (w0, nw) in l2chunks:
        for q in range(4):
            col2_off.append(off)
            off += (nw * 128) // 16
    GL2 = off
    gl2 = np.empty((NC, 16, GL2), np.int16)
    k = 0
    for (w0, nw) in l2chunks:
        for q in range(4):
            o = col2_off[k]
            n16 = (nw * 128) // 16
            for c in range(NC):
                flat = l2[c, q, w0 * 128:(w0 + nw) * 128].astype(np.int16)
                gl2[c][:, o:o + n16] = flat.reshape(-1, 16).T
            k += 1

    # per-core dense inputs
    xT = np.zeros((NC, IN_CH, SHP), np.float32)
    dinvs = np.zeros((NC, 128, NW), np.float32)
    for c in range(NC):
        xT[c, :, :SH] = x[node_at[c, :SH]].T
        dv = np.zeros(SHP, np.float32)
        dv[:SH] = dinv[node_at[c, :SH]]
        dinvs[c] = dv.reshape(NW, 128).T

    sched = dict(
        Rs=Rs, cumR=cumR, l1chunks=l1chunks, col_off=col_off, GL1=GL1,
        l2chunks=l2chunks, col2_off=col2_off, GL2=GL2,
    )
    arrays = dict(xT=xT, dinvs=dinvs, gell=gell, gl2=gl2, node_at=node_at)
    return sched, arrays


def _sched_key(sched):
    import hashlib
    h = hashlib.sha256()
    h.update(sched["Rs"].tobytes())
    h.update(np.asarray(sched["l1chunks"], np.int64).tobytes())
    h.update(np.asarray(sched["l2chunks"], np.int64).tobytes())
    return h.hexdigest()


# ----------------------------------------------------------------------------
# bass module
# ----------------------------------------------------------------------------

def _build(sched, reps=1, variant="full"):
    Relu = mybir.ActivationFunctionType.Relu
    Copy = mybir.ActivationFunctionType.Copy
    X = mybir.AxisListType.X
    ADD = mybir.AluOpType.add
    MULT = mybir.AluOpType.mult

    Rs, cumR = sched["Rs"], sched["cumR"]
    l1chunks, col_off = sched["l1chunks"], sched["col_off"]
    l2chunks, col2_off = sched["l2chunks"], sched["col2_off"]
    GL1, GL2 = sched["GL1"], sched["GL2"]
    max_ns = max(ch[4] for ch in l1chunks)
    max_l1w = max(ch[2] for ch in l1chunks)

    nc = bacc.Bacc("TRN2", target_bir_lowering=False, debug=False,
                   num_devices=NC)

    xT_d = nc.dram_tensor("xT", [IN_CH, SHP], f32, kind="ExternalInput")
    w1_d = nc.dram_tensor("w1", [IN_CH, HID], f32, kind="ExternalInput")
    w23_d = nc.dram_tensor("w23", [HID, 2 * EMB], f32, kind="ExternalInput")
    b1_d = nc.dram_tensor("b1r", [128, HID], f32, kind="ExternalInput")
    b23_d = nc.dram_tensor("b23r", [128, 2 * EMB], f32, kind="ExternalInput")
    dinv_d = nc.dram_tensor("dinvs", [128, NW], f32, kind="ExternalInput")
    gell_d = nc.dram_tensor("gell", [128, GL1], i16, kind="ExternalInput")
    gl2_d = nc.dram_tensor("gl2", [128, GL2], i16, kind="ExternalInput")
    out_d = nc.dram_tensor("out", [SHP, HID], f32, kind="ExternalOutput")

    with tile.TileContext(nc) as tc:
        tc.race_detector_enabled = False
        with (
            tc.tile_pool(name="const", bufs=1) as cpool,
            tc.tile_pool(name="xslab", bufs=2) as xpool,
            tc.tile_pool(name="stage", bufs=2) as spool,
            tc.tile_pool(name="g1", bufs=2) as gpool,
            tc.tile_pool(name="acc", bufs=3) as apool,
            tc.tile_pool(name="g2", bufs=2) as g2pool,
            tc.tile_pool(name="red", bufs=2) as rpool,
            tc.tile_pool(name="hts", bufs=3) as hpool,
            tc.tile_pool(name="psA", bufs=2, space="PSUM") as ppA,
            tc.tile_pool(name="psT", bufs=2, space="PSUM") as ppT,
            tc.tile_pool(name="psZ", bufs=2, space="PSUM") as ppZ,
            tc.tile_pool(name="dram", bufs=1, space="DRAM") as dpool,
        ):
            w1s = cpool.tile([IN_CH, HID], f32)
            nc.sync.dma_start(out=w1s[:, :], in_=w1_d[:, :])
            w23s = cpool.tile([HID, 2 * EMB], f32)
            nc.sync.dma_start(out=w23s[:, :], in_=w23_d[:, :])
            b1s = cpool.tile([128, HID], f32)
            nc.sync.dma_start(out=b1s[:, :], in_=b1_d[:, :])
            b23s = cpool.tile([128, 2 * EMB], f32)
            nc.sync.dma_start(out=b23s[:, :], in_=b23_d[:, :])
            dinvs = cpool.tile([128, NW], f32)
            nc.sync.dma_start(out=dinvs[:, :], in_=dinv_d[:, :])
            gells = cpool.tile([128, GL1], i16)
            nc.sync.dma_start(out=gells[:, :], in_=gell_d[:, :])
            gl2s = cpool.tile([128, GL2], i16)
            nc.sync.dma_start(out=gl2s[:, :], in_=gl2_d[:, :])
            ident = cpool.tile([128, 128], f32)
            make_identity(nc, ident[:, :])

            u1c = dpool.tile([SHP, HID], f32, tag="u1c")
            u2c = dpool.tile([SHP, HID], f32, tag="u2c")
            table1 = dpool.tile([NC * SHP, HID], f32, tag="table1")
            table2 = dpool.tile([NC * SHP, HID], f32, tag="table2")
            Y = [[dpool.tile([SHP, HID], f32, name=f"Y{p}{q}", tag=f"Y{p}{q}")
                  for q in range(4)] for p in range(2)]
            # Pre-assign DRAM addresses: the scheduling-time trace sim
            # otherwise sees every Shared tile at addr 0 and flags the two
            # collectives as a double write to one tensor.
            from concourse.tile_scheduler import allocate_dram_tiles
            allocate_dram_tiles(tc.tiles, nc)

            def propagation(prop, table, bias, relu, dest):
                # level 1: per-quarter ELL gather + window reduce -> Y
                for (q, w0, nwc, s0, ns) in l1chunks:
                    ci = l1chunks.index((q, w0, nwc, s0, ns))
                    n = ns * 128
                    G = gpool.tile([128, max_ns, HID], f32, tag="g1")
                    nc.gpsimd.dma_gather(
                        G[:, :ns, :],
                        table[q * QROWS:(q + 1) * QROWS, :],
                        gells[:, col_off[ci]:col_off[ci] + n // 16],
                        n, n, HID, single_packet=False,
                    )
                    accc = apool.tile([128, max_l1w * HID], f32, tag="acc")
                    for i in range(nwc):
                        off = int(cumR[q, w0 + i] - s0)
                        R = int(Rs[q, w0 + i])
                        nc.vector.tensor_reduce(
                            out=accc[:, i * HID:(i + 1) * HID],
                            in_=G[:, off:off + R, :].transpose([0, 2, 1]),
                            axis=X, op=ADD,
                        )
                    nc.sync.dma_start(
                        out=Y[prop][q][w0 * 128:(w0 + nwc) * 128, :]
                            .rearrange("(a p) c -> p a c", p=128),
                        in_=accc[:, :nwc * HID]
                            .rearrange("p (a c) -> p a c", c=HID),
                    )

                # level 2: combine quarters, epilogue
                k = 0
                for (w0, nw) in l2chunks:
                    G2 = g2pool.tile([128, 4, L2_CHUNK, HID], f32, tag="g2")
                    for q in range(4):
                        n = nw * 128
                        nc.gpsimd.dma_gather(
                            G2[:, q, :nw, :],
                            Y[prop][q][:, :],
                            gl2s[:, col2_off[k]:col2_off[k] + n // 16],
                            n, n, HID, single_packet=False,
                        )
                        k += 1
                    red = rpool.tile([128, L2_CHUNK * HID], f32, tag="red")
                    nc.vector.tensor_reduce(
                        out=red[:, :nw * HID],
                        in_=G2[:, :, :nw, :].transpose([0, 2, 3, 1]),
                        axis=X, op=ADD,
                    )
                    rv = red[:, :nw * HID].rearrange("p (a c) -> p a c", c=HID)
                    nc.vector.tensor_tensor(
                        out=rv, in0=rv,
                        in1=dinvs[:, w0:w0 + nw].unsqueeze(2)
                            .to_broadcast([128, nw, HID]),
                        op=MULT,
                    )
                    nc.vector.tensor_tensor(
                        out=rv, in0=rv,
                        in1=bias[:, :].unsqueeze(1)
                            .to_broadcast([128, nw, HID]),
                        op=ADD,
                    )
                    if relu:
                        h = hpool.tile([128, L2_CHUNK * HID], f32, tag="h")
                        nc.scalar.activation(
                            out=h[:, :nw * HID], in_=red[:, :nw * HID],
                            func=Relu)
                        ust = spool.tile([128, L2_CHUNK, HID], f32, tag="ust")
                        for i in range(nw):
                            hT = ppT.tile([HID, 128], f32, tag="psT")
                            nc.tensor.transpose(
                                out=hT[:, :],
                                in_=h[:, i * HID:(i + 1) * HID],
                                identity=ident[:, :])
                            hTs = hpool.tile([HID, 128], f32, tag="hts")
                            nc.scalar.copy(out=hTs[:, :], in_=hT[:, :])
                            z2 = ppZ.tile([128, 2 * EMB], f32, tag="psZ")
                            nc.tensor.matmul(
                                out=z2[:, :], lhsT=hTs[:, :], rhs=w23s[:, :],
                                start=True, stop=True)
                            nc.scalar.activation(
                                out=ust[:, i, :], in_=z2[:, :], func=Copy,
                                scale=dinvs[:, w0 + i:w0 + i + 1])
                        nc.sync.dma_start(
                            out=dest[w0 * 128:(w0 + nw) * 128, :]
                                .rearrange("(a p) c -> p a c", p=128),
                            in_=ust[:, :nw, :],
                        )
                    else:
                        nc.sync.dma_start(
                            out=dest[w0 * 128:(w0 + nw) * 128, :]
                                .rearrange("(a p) c -> p a c", p=128),
                            in_=rv,
                        )

            for _rep in range(reps):
                # phase A: u1 = dinv * (x @ W1), shard-local
                TPS = 14  # tiles per slab
                for sl in range(NW // TPS):
                    xsl = xpool.tile([IN_CH, TPS * 128], f32, tag="xslab")
                    nc.sync.dma_start(
                        out=xsl[:, :],
                        in_=xT_d[:, sl * TPS * 128:(sl + 1) * TPS * 128])
                    st = spool.tile([128, TPS, HID], f32, tag="stA")
                    for i in range(TPS):
                        t = sl * TPS + i
                        ps = ppA.tile([128, HID], f32, tag="psA")
                        nc.tensor.matmul(
                            out=ps[:, :], lhsT=xsl[:, i * 128:(i + 1) * 128],
                            rhs=w1s[:, :], start=True, stop=True)
                        nc.vector.tensor_scalar_mul(
                            st[:, i, :], ps[:, :], dinvs[:, t:t + 1])
                    nc.sync.dma_start(
                        out=u1c[sl * TPS * 128:(sl + 1) * TPS * 128, :]
                            .rearrange("(a p) c -> p a c", p=128),
                        in_=st[:, :, :],
                    )

                if variant != "nocc":
                    nc.gpsimd.collective_compute(
                        "AllGather", mybir.AluOpType.bypass,
                        ins=[u1c[:, :]], outs=[table1[:, :]],
                        replica_groups=[list(range(NC))],
                    )
                else:
                    for c in range(NC):
                        nc.sync.dma_start(
                            out=table1[c * SHP:(c + 1) * SHP, :], in_=u1c[:, :])
                if variant != "nog":
                    propagation(0, table1, b1s, True, u2c)
                else:
                    nc.sync.dma_start(out=u2c[:, :], in_=table1[0:SHP, :])
                if variant != "nocc":
                    nc.gpsimd.collective_compute(
                        "AllGather", mybir.AluOpType.bypass,
                        ins=[u2c[:, :]], outs=[table2[:, :]],
                        replica_groups=[list(range(NC))],
                    )
                else:
                    for c in range(NC):
                        nc.sync.dma_start(
                            out=table2[c * SHP:(c + 1) * SHP, :], in_=u2c[:, :])
                if variant != "nog":
                    propagation(1, table2, b23s, False, out_d)
                else:
                    nc.sync.dma_start(
                        out=out_d[:, :], in_=table2[0:SHP, :])

    nc.compile()
    return nc


_CACHE = {}


def _get_module(sched, reps=1, variant="full"):
    key = (_sched_key(sched), reps, variant)
    if key not in _CACHE:
        _CACHE[key] = _build(sched, reps, variant)
    return _CACHE[key]


def _in_maps(arrays, W1, b1, W2, b2, W3, b3):
    W1 = np.asarray(W1, np.float32)
    w23 = np.concatenate(
        [np.asarray(W2, np.float32), np.asarray(W3, np.float32)], axis=1)
    b1r = np.tile(np.asarray(b1, np.float32)[None, :], (128, 1))
    b23r = np.tile(
        np.concatenate([np.asarray(b2, np.float32),
                        np.asarray(b3, np.float32)])[None, :], (128, 1))
    maps = []
    for c in range(NC):
        maps.append({
            "xT": np.ascontiguousarray(arrays["xT"][c]),
            "w1": W1,
            "w23": w23,
            "b1r": b1r,
            "b23r": b23r,
            "dinvs": np.ascontiguousarray(arrays["dinvs"][c]),
            "gell": np.ascontiguousarray(np.tile(arrays["gell"][c], (8, 1))),
            "gl2": np.ascontiguousarray(np.tile(arrays["gl2"][c], (8, 1))),
        })
    return maps


def kernel(x, edges, W1, b1, W2, b2, W3, b3):
    sched, arrays = _prepare(x, edges)
    nc = _get_module(sched, reps=1)
    maps = _in_maps(arrays, W1, b1, W2, b2, W3, b3)
    res = bass_utils.run_bass_kernel_spmd(nc, maps, core_ids=list(range(NC)))
    out2 = np.empty((N, EMB), np.float32)
    out3 = np.empty((N, EMB), np.float32)
    node_at = arrays["node_at"]
    for c in range(NC):
        r = res.results[c]["out"]
        out2[node_at[c, :SH]] = r[:SH, :EMB]
        out3[node_at[c, :SH]] = r[:SH, EMB:]
    return out2, out3


def benchmark(x, edges, W1, b1, W2, b2, W3, b3, reps=4, iters=5,
              variant="full", verbose=False):
    """Estimate on-HW exec time via the rep-delta trick: identical NEFF,
    body repeated `reps` times; transfers are identical, so
    (wall(reps) - wall(1)) / (reps - 1) ~= one body execution."""
    sched, arrays = _prepare(x, edges)
    maps = _in_maps(arrays, W1, b1, W2, b2, W3, b3)
    nc1 = _get_module(sched, reps=1, variant=variant)
    ncR = _get_module(sched, reps=reps, variant=variant)
    cores = list(range(NC))

    def timeit(mod):
        ts = []
        for _ in range(iters):
            t0 = time.time()
            bass_utils.run_bass_kernel_spmd(mod, maps, core_ids=cores)
            ts.append(time.time() - t0)
        return ts

    timeit(nc1)[:1]  # warm
    t1 = timeit(nc1)
    tR = timeit(ncR)
    if verbose:
        print(f"[{variant}] t1={['%.3f'%t for t in t1]} tR={['%.3f'%t for t in tR]}")
    return (min(tR) - min(t1)) / (reps - 1) * 1e9


# revision 9
# speedup vs baseline: 1.3634x; 1.1002x over previous
"""GCN encoder (3x GCNConv, cached symmetric norm) on 8 Trainium2 NeuronCores.

Instruction-lean revision: the axon-virtualized cores pay ~25-80us per
instruction, so everything is batched into few, large instructions:
  - level-1 ELL gathers use a uniform R per chunk -> ONE strided reduce per
    chunk instead of one per window
  - level-2 combines 4 quarter-partials with one gather per (chunk, quarter)
    and ONE 4D reduce per chunk
  - matmuls accumulate 8 windows into one PSUM bank, copied out once
  - dinv is folded into h before z2 so no per-window scaling is needed
"""

import time
import numpy as np

import concourse.bass as bass
import concourse.tile as tile
from concourse import bacc, mybir
from concourse import bass_utils
from concourse.masks import make_identity

N = 100000
IN_CH, HID, EMB = 128, 64, 32
NC = 8
SH = 12500
SHP = 12544
NW = 98
QROWS = 2 * SHP
PADIDX = 12500
L2_CHUNK = 25
L1_SLOT_BUDGET = 112
L1_MAX_WIN = 32

f32 = mybir.dt.float32
i16 = mybir.dt.int16


def _prepare(x, edges):
    x = np.asarray(x, dtype=np.float32)
    src = np.asarray(edges[0], dtype=np.int64)
    dst = np.asarray(edges[1], dtype=np.int64)

    allsrc = np.concatenate([src, np.arange(N, dtype=np.int64)])
    alldst = np.concatenate([dst, np.arange(N, dtype=np.int64)])
    deg = np.bincount(alldst, minlength=N)
    dinv = (1.0 / np.sqrt(deg.astype(np.float64))).astype(np.float32)

    equar = allsrc // (2 * SH)
    ecore = alldst // SH
    kdq = np.bincount(alldst * 4 + equar, minlength=4 * N).reshape(N, 4)

    rank = np.empty(N, np.int64)
    node_at = np.full((NC, SHP), -1, np.int64)
    for c in range(NC):
        nodes = np.arange(c * SH, (c + 1) * SH)
        order = np.argsort(-deg[nodes], kind="stable")
        rank[nodes[order]] = np.arange(SH)
        node_at[c, :SH] = nodes[order]

    rank_q = np.empty((4, N), np.int64)
    Rs = np.zeros((4, NW), np.int64)
    for c in range(NC):
        nodes = np.arange(c * SH, (c + 1) * SH)
        for q in range(4):
            order = np.argsort(-kdq[nodes, q], kind="stable")
            rank_q[q, nodes[order]] = np.arange(SH)
            kk = kdq[nodes[order], q]
            Rs[q] = np.maximum(Rs[q], kk[::128][:NW])
    Rs = np.maximum(Rs, 1)

    # level-1 chunks with uniform R per chunk (windows are degree-sorted so
    # R is non-increasing; chunk R = first window's R)
    l1chunks = []  # (q, w0, nw, Rc, slot_off)
    for q in range(4):
        w = 0
        off = 0
        while w < NW:
            Rc = int(Rs[q, w])
            w0 = w
            while (w < NW and (w - w0) < L1_MAX_WIN
                   and (w - w0 + 1) * Rc <= L1_SLOT_BUDGET):
                w += 1
            l1chunks.append((q, w0, w - w0, Rc, off))
            off += (w - w0) * Rc

    # ELL fill: chunk-local slot = (w - w0)*Rc + r, gather pos = slot*128 + p
    slot_off_of = {}
    Rc_of_w = np.zeros((4, NW), np.int64)
    chunk_base = np.zeros((4, NW), np.int64)
    for (q, w0, nw, Rc, off) in l1chunks:
        for i in range(nw):
            Rc_of_w[q, w0 + i] = Rc
            chunk_base[q, w0 + i] = off + i * Rc
    total_slots = sum(nw * Rc for (_q, _w, nw, Rc, _o) in l1chunks) // 4 * 4
    SL = sum(nw * Rc for (_q, _w0, nw, Rc, _o) in l1chunks if _q == 0)
    SLs = [sum(nw * Rc for (qq, _w0, nw, Rc, _o) in l1chunks if qq == q)
           for q in range(4)]

    tloc = ((np.arange(N) // SH) % 2) * SHP + rank

    rq_of_msg = rank_q[equar, alldst]
    sortkey = (ecore * 4 + equar) * SHP + rq_of_msg
    ordix = np.argsort(sortkey, kind="stable")
    sk = sortkey[ordix]
    starts = np.r_[0, np.flatnonzero(np.diff(sk)) + 1]
    counts = np.diff(np.r_[starts, len(sk)])
    slot = np.arange(len(sk)) - np.repeat(starts, counts)
    ms = allsrc[ordix]
    rq_s = rq_of_msg[ordix]
    w_of = rq_s // 128
    p_of = rq_s % 128
    c_of = ecore[ordix]
    q_of = equar[ordix]
    rowpos = chunk_base[q_of, w_of] + slot
    vals = tloc[ms].astype(np.int16)

    ell = [[np.full((SLs[q], 128), PADIDX, np.int16) for q in range(4)]
           for _ in range(NC)]
    for c in range(NC):
        mc = c_of == c
        for q in range(4):
            m = mc & (q_of == q)
            ell[c][q][rowpos[m], p_of[m]] = vals[m]

    # pack level-1 idx segments (per chunk) into [16, GL1]
    col_off = []
    off = 0
    for (q, w0, nw, Rc, soff) in l1chunks:
        col_off.append(off)
        off += (nw * Rc * 128) // 16
    GL1 = off
    gell = np.empty((NC, 16, GL1), np.int16)
    for c in range(NC):
        for ci, (q, w0, nw, Rc, soff) in enumerate(l1chunks):
            ns = nw * Rc
            flat = ell[c][q][soff:soff + ns, :].reshape(-1)
            gell[c][:, col_off[ci]:col_off[ci] + ns * 8] = \
                flat.reshape(-1, 16).T

    # level-2
    l2 = np.full((NC, 4, SHP), PADIDX, np.int64)
    for c in range(NC):
        real = node_at[c, :SH]
        for q in range(4):
            l2[c, q, :SH] = rank_q[q, real]
    l2chunks = []
    w = 0
    while w < NW:
        nw = min(L2_CHUNK, NW - w)
        l2chunks.append((w, nw))
        w += nw
    col2_off = []
    off = 0
    for (w0, nw) in l2chunks:
        for q in range(4):
            col2_off.append(off)
            off += (nw * 128) // 16
    GL2 = off
    gl2 = np.empty((NC, 16, GL2), np.int16)
    k = 0
    for (w0, nw) in l2chunks:
        for q in range(4):
            o = col2_off[k]
            for c in range(NC):
                flat = l2[c, q, w0 * 128:(w0 + nw) * 128].astype(np.int16)
                gl2[c][:, o:o + nw * 8] = flat.reshape(-1, 16).T
            k += 1

    xT = np.zeros((NC, IN_CH, SHP), np.float32)
    dinvs = np.zeros((NC, 128, NW), np.float32)
    for c in range(NC):
        xT[c, :, :SH] = x[node_at[c, :SH]].T
        dv = np.zeros(SHP, np.float32)
        dv[:SH] = dinv[node_at[c, :SH]]
        dinvs[c] = dv.reshape(NW, 128).T

    sched = dict(Rs=Rs, l1chunks=l1chunks, col_off=col_off, GL1=GL1,
                 l2chunks=l2chunks, col2_off=col2_off, GL2=GL2)
    arrays = dict(xT=xT, dinvs=dinvs, gell=gell, gl2=gl2, node_at=node_at)
    return sched, arrays


def _sched_key(sched):
    import hashlib
    h = hashlib.sha256()
    h.update(sched["Rs"].tobytes())
    h.update(np.asarray(sched["l1chunks"], np.int64).tobytes())
    h.update(np.asarray(sched["l2chunks"], np.int64).tobytes())
    return h.hexdigest()


def _build(sched, reps=1, variant="full"):
    Relu = mybir.ActivationFunctionType.Relu
    X = mybir.AxisListType.X
    ADD = mybir.AluOpType.add
    MULT = mybir.AluOpType.mult

    l1chunks, col_off = sched["l1chunks"], sched["col_off"]
    l2chunks, col2_off = sched["l2chunks"], sched["col2_off"]
    GL1, GL2 = sched["GL1"], sched["GL2"]
    max_ns = max(nw * Rc for (_q, _w, nw, Rc, _o) in l1chunks)
    max_l2 = max(nw for (_w, nw) in l2chunks)

    nc = bacc.Bacc("TRN2", target_bir_lowering=False, debug=False,
                   num_devices=NC)

    xT_d = nc.dram_tensor("xT", [IN_CH, SHP], f32, kind="ExternalInput")
    w1_d = nc.dram_tensor("w1", [IN_CH, HID], f32, kind="ExternalInput")
    w23_d = nc.dram_tensor("w23", [2 * HID, 2 * EMB], f32, kind="ExternalInput")
    b1_d = nc.dram_tensor("b1r", [128, HID], f32, kind="ExternalInput")
    b23_d = nc.dram_tensor("b23r", [128, 2 * EMB], f32, kind="ExternalInput")
    dinv_d = nc.dram_tensor("dinvs", [128, NW], f32, kind="ExternalInput")
    gell_d = nc.dram_tensor("gell", [128, GL1], i16, kind="ExternalInput")
    gl2_d = nc.dram_tensor("gl2", [128, GL2], i16, kind="ExternalInput")
    out_d = nc.dram_tensor("out", [SHP, HID], f32, kind="ExternalOutput")

    with tile.TileContext(nc) as tc:
        with (
            tc.tile_pool(name="const", bufs=1) as cpool,
            tc.tile_pool(name="xslab", bufs=2) as xpool,
            tc.tile_pool(name="stage", bufs=2) as spool,
            tc.tile_pool(name="g1", bufs=2) as gpool,
            tc.tile_pool(name="g2", bufs=1) as g2pool,
            tc.tile_pool(name="red", bufs=2) as rpool,
            tc.tile_pool(name="hts", bufs=4) as hpool,
            tc.tile_pool(name="psA", bufs=2, space="PSUM") as ppA,
            tc.tile_pool(name="psT", bufs=2, space="PSUM") as ppT,
            tc.tile_pool(name="psZ", bufs=2, space="PSUM") as ppZ,
            tc.tile_pool(name="dram", bufs=1, space="DRAM") as dpool,
        ):
            w1s = cpool.tile([IN_CH, HID], f32)
            nc.sync.dma_start(out=w1s[:, :], in_=w1_d[:, :])
            w23s = cpool.tile([2 * HID, 2 * EMB], f32)
            nc.sync.dma_start(out=w23s[:, :], in_=w23_d[:, :])
            b1s = cpool.tile([128, HID], f32)
            nc.sync.dma_start(out=b1s[:, :], in_=b1_d[:, :])
            b23s = cpool.tile([128, 2 * EMB], f32)
            nc.sync.dma_start(out=b23s[:, :], in_=b23_d[:, :])
            dinvs = cpool.tile([128, NW], f32)
            nc.sync.dma_start(out=dinvs[:, :], in_=dinv_d[:, :])
            gl2s = cpool.tile([128, GL2], i16)
            nc.sync.dma_start(out=gl2s[:, :], in_=gl2_d[:, :])
            ident = cpool.tile([128, 128], f32)
            make_identity(nc, ident[:, :])

            u1c = dpool.tile([SHP, HID], f32, tag="u1c")
            u2c = dpool.tile([SHP, HID], f32, tag="u2c")
            table1 = dpool.tile([NC * SHP, HID], f32, tag="table1")
            table2 = dpool.tile([NC * SHP, HID], f32, tag="table2")
            Y = [[dpool.tile([SHP, HID], f32, name=f"Y{p}{q}", tag=f"Y{p}{q}")
                  for q in range(4)] for p in range(2)]
            from concourse.tile_scheduler import allocate_dram_tiles
            allocate_dram_tiles(tc.tiles, nc)

            def propagation(prop, table, bias, relu, dest):
                # level 1
                for ci, (q, w0, nwc, Rc, soff) in enumerate(l1chunks):
                    ns = nwc * Rc
                    n = ns * 128
                    idxt = hpool.tile([128, max_ns * 8], i16, tag="l1idx",
                                      name="l1idx")
                    nc.sync.dma_start(
                        out=idxt[:, :ns * 8],
                        in_=gell_d[:, col_off[ci]:col_off[ci] + ns * 8])
                    G = gpool.tile([128, max_ns, HID], f32, tag="g1",
                                   name="g1t")
                    nc.gpsimd.dma_gather(
                        G[:, :ns, :],
                        table[q * QROWS:(q + 1) * QROWS, :],
                        idxt[:, :ns * 8],
                        n, n, HID, single_packet=False,
                    )
                    red = rpool.tile([128, L1_MAX_WIN * HID], f32, tag="l1r",
                                     name="l1r")
                    nc.vector.tensor_reduce(
                        out=red[:, :nwc * HID],
                        in_=G[:, :ns, :]
                            .rearrange("p (w r) c -> p w r c", r=Rc)
                            .transpose([0, 1, 3, 2]),
                        axis=X, op=ADD,
                    )
                    nc.sync.dma_start(
                        out=Y[prop][q][w0 * 128:(w0 + nwc) * 128, :]
                            .rearrange("(a p) c -> p a c", p=128),
                        in_=red[:, :nwc * HID]
                            .rearrange("p (a c) -> p a c", c=HID),
                    )

                # level 2 + epilogue (+ z2 for prop 0)
                k = 0
                for (w0, nw) in l2chunks:
                    G2 = g2pool.tile([128, 4, max_l2, HID], f32, tag="g2",
                                     name="g2t")
                    for q in range(4):
                        n = nw * 128
                        nc.gpsimd.dma_gather(
                            G2[:, q, :nw, :],
                            Y[prop][q][:, :],
                            gl2s[:, col2_off[k]:col2_off[k] + nw * 8],
                            n, n, HID, single_packet=False,
                        )
                        k += 1
                    red = rpool.tile([128, L2_CHUNK * HID], f32, tag="l2r",
                                     name="l2r")
                    nc.vector.tensor_reduce(
                        out=red[:, :nw * HID],
                        in_=G2[:, :, :nw, :].transpose([0, 2, 3, 1]),
                        axis=X, op=ADD,
                    )
                    rv = red[:, :nw * HID].rearrange("p (a c) -> p a c", c=HID)
                    nc.vector.tensor_tensor(
                        out=rv, in0=rv,
                        in1=dinvs[:, w0:w0 + nw].unsqueeze(2)
                            .to_broadcast([128, nw, HID]),
                        op=MULT)
                    nc.vector.tensor_tensor(
                        out=rv, in0=rv,
                        in1=bias[:, :].unsqueeze(1)
                            .to_broadcast([128, nw, HID]),
                        op=ADD)
                    if relu:
                        # h' = dinv * relu(y); u2 = h' @ W23
                        h = hpool.tile([128, L2_CHUNK * HID], f32, tag="h",
                                       name="ht")
                        nc.scalar.activation(out=h[:, :nw * HID],
                                             in_=red[:, :nw * HID], func=Relu)
                        nc.vector.tensor_tensor(
                            out=h[:, :nw * HID]
                                .rearrange("p (a c) -> p a c", c=HID),
                            in0=h[:, :nw * HID]
                                .rearrange("p (a c) -> p a c", c=HID),
                            in1=dinvs[:, w0:w0 + nw].unsqueeze(2)
                                .to_broadcast([128, nw, HID]),
                            op=MULT)
                        ust = spool.tile([128, L2_CHUNK, HID], f32, tag="ust",
                                         name="ust")
                        g0 = 0
                        while g0 < nw:
                            gn = min(8, nw - g0)  # windows in this group
                            z2 = ppZ.tile([128, 8 * HID], f32, tag="psZ",
                                          name="psZt")
                            for j in range(gn):
                                hT = ppT.tile([HID, 128], f32, tag="psT",
                                              name="psTt")
                                nc.tensor.transpose(
                                    out=hT[:, :],
                                    in_=h[:, (g0 + j) * HID:
                                          (g0 + j + 1) * HID],
                                    identity=ident[:, :])
                                hc = hpool.tile([HID, 128], f32, tag="hts",
                                                name="htst")
                                nc.scalar.copy(out=hc[:, :], in_=hT[:, :])
                                nc.tensor.matmul(
                                    out=z2[:, j * HID:(j + 1) * HID],
                                    lhsT=hc[:, :],
                                    rhs=w23s[0:HID, :],
                                    start=True, stop=True)
                            nc.vector.tensor_copy(
                                out=ust[:, g0:g0 + gn, :],
                                in_=z2[:, :gn * HID]
                                    .rearrange("p (a c) -> p a c", c=HID))
                            g0 += gn
                        nc.sync.dma_start(
                            out=dest[w0 * 128:(w0 + nw) * 128, :]
                                .rearrange("(a p) c -> p a c", p=128),
                            in_=ust[:, :nw, :])
                    else:
                        nc.sync.dma_start(
                            out=dest[w0 * 128:(w0 + nw) * 128, :]
                                .rearrange("(a p) c -> p a c", p=128),
                            in_=rv)

            for _rep in range(reps):
                # phase A: u1 = dinv * (x @ W1)
                TPS = 14
                for sl in range(NW // TPS):
                    xsl = xpool.tile([IN_CH, TPS * 128], f32, tag="xslab",
                                     name="xslt")
                    nc.sync.dma_start(
                        out=xsl[:, :],
                        in_=xT_d[:, sl * TPS * 128:(sl + 1) * TPS * 128])
                    st = spool.tile([128, TPS, HID], f32, tag="stA",
                                    name="stAt")
                    for g in range(2):  # 7-window PSUM groups
                        ps = ppA.tile([128, 7 * HID], f32, tag="psA",
                                      name="psAt")
                        for i in range(7):
                            t = g * 7 + i
                            nc.tensor.matmul(
                                out=ps[:, i * HID:(i + 1) * HID],
                                lhsT=xsl[:, t * 128:(t + 1) * 128],
                                rhs=w1s[:, :], start=True, stop=True)
                        tw = sl * TPS + g * 7
                        nc.vector.tensor_tensor(
                            out=st[:, g * 7:(g + 1) * 7, :],
                            in0=ps[:, :].rearrange("p (a c) -> p a c", c=HID),
                            in1=dinvs[:, tw:tw + 7].unsqueeze(2)
                                .to_broadcast([128, 7, HID]),
                            op=MULT)
                    nc.sync.dma_start(
                        out=u1c[sl * TPS * 128:(sl + 1) * TPS * 128, :]
                            .rearrange("(a p) c -> p a c", p=128),
                        in_=st[:, :, :])

                if variant != "nocc":
                    nc.gpsimd.collective_compute(
                        "AllGather", mybir.AluOpType.bypass,
                        ins=[u1c[:, :]], outs=[table1[:, :]],
                        replica_groups=[list(range(NC))])
                else:
                    for c in range(NC):
                        nc.sync.dma_start(
                            out=table1[c * SHP:(c + 1) * SHP, :],
                            in_=u1c[:, :])
                if variant != "nog":
                    propagation(0, table1, b1s, True, u2c)
                else:
                    nc.sync.dma_start(out=u2c[:, :], in_=table1[0:SHP, :])
                if variant != "nocc":
                    nc.gpsimd.collective_compute(
                        "AllGather", mybir.AluOpType.bypass,
                        ins=[u2c[:, :]], outs=[table2[:, :]],
                        replica_groups=[list(range(NC))])
                else:
                    for c in range(NC):
                        nc.sync.dma_start(
                            out=table2[c * SHP:(c + 1) * SHP, :],
                            in_=u2c[:, :])
                if variant != "nog":
                    propagation(1, table2, b23s, False, out_d)
                else:
                    nc.sync.dma_start(out=out_d[:, :], in_=table2[0:SHP, :])

    nc.compile()
    return nc


_CACHE = {}


def _get_module(sched, reps=1, variant="full"):
    key = (_sched_key(sched), reps, variant)
    if key not in _CACHE:
        _CACHE[key] = _build(sched, reps, variant)
    return _CACHE[key]


def _in_maps(arrays, W1, b1, W2, b2, W3, b3):
    W1 = np.asarray(W1, np.float32)
    w23 = np.concatenate(
        [np.asarray(W2, np.float32), np.asarray(W3, np.float32)], axis=1)
    w23 = np.tile(w23, (2, 1))
    b1r = np.tile(np.asarray(b1, np.float32)[None, :], (128, 1))
    b23r = np.tile(
        np.concatenate([np.asarray(b2, np.float32),
                        np.asarray(b3, np.float32)])[None, :], (128, 1))
    maps = []
    for c in range(NC):
        maps.append({
            "xT": np.ascontiguousarray(arrays["xT"][c]),
            "w1": W1, "w23": w23, "b1r": b1r, "b23r": b23r,
            "dinvs": np.ascontiguousarray(arrays["dinvs"][c]),
            "gell": np.ascontiguousarray(np.tile(arrays["gell"][c], (8, 1))),
            "gl2": np.ascontiguousarray(np.tile(arrays["gl2"][c], (8, 1))),
        })
    return maps


def kernel(x, edges, W1, b1, W2, b2, W3, b3):
    sched, arrays = _prepare(x, edges)
    nc = _get_module(sched, reps=1)
    maps = _in_maps(arrays, W1, b1, W2, b2, W3, b3)
    res = bass_utils.run_bass_kernel_spmd(nc, maps, core_ids=list(range(NC)))
    out2 = np.empty((N, EMB), np.float32)
    out3 = np.empty((N, EMB), np.float32)
    node_at = arrays["node_at"]
    for c in range(NC):
        r = res.results[c]["out"]
        out2[node_at[c, :SH]] = r[:SH, :EMB]
        out3[node_at[c, :SH]] = r[:SH, EMB:]
    return out2, out3


def benchmark(x, edges, W1, b1, W2, b2, W3, b3, reps=4, iters=5,
              variant="full", verbose=False):
    sched, arrays = _prepare(x, edges)
    maps = _in_maps(arrays, W1, b1, W2, b2, W3, b3)
    nc1 = _get_module(sched, reps=1, variant=variant)
    ncR = _get_module(sched, reps=reps, variant=variant)
    cores = list(range(NC))

    def timeit(mod):
        ts = []
        for _ in range(iters):
            t0 = time.time()
            bass_utils.run_bass_kernel_spmd(mod, maps, core_ids=cores)
            ts.append(time.time() - t0)
        return ts

    timeit(nc1)[:1]
    t1 = timeit(nc1)
    tR = timeit(ncR)
    if verbose:
        print(f"[{variant}] t1={['%.3f' % t for t in t1]} "
              f"tR={['%.3f' % t for t in tR]}")
    return (min(tR) - min(t1)) / (reps - 1) * 1e9
